# revision 29
# baseline (speedup 1.0000x reference)
"""
Trainium2 Bass kernel for AlphaFold-style gated MSA attention.

  out[b] = (softmax(qk^T/sqrt(hd) + bias[b] + nb) @ v * sigmoid(gate)) @ Wo + bo

Shapes (hardcoded): B=64, Q=K=512, C=256, H=8, HD=32, OUT=256.
Sharding: data-parallel over batch, 8 batches per core on 8 NeuronCores.

Per-core dataflow (everything in "transposed" [channel, seq] layouts):
  - projections:  qT/kT [hc, q] and v [k, hc], gate-logits [hc, q]
    (float32r matmuls: full-rate fp32)
  - logits^T[k,q] per head via row-tiled (K=32) matmuls, 4 heads concurrent
  - biases pre-transposed AND pre-combined on the host (bf16 s12), then
    accumulated into logits in PSUM via identity-matmul on PE (most
    head-pairs) or added on DVE on the way out of PSUM (the rest)
  - exp on ScalarE, PSUM->SBUF, bf16 out (no max subtraction needed:
    |logits| <~ 12 so exp is safely in range)
  - AV and softmax denominator: col-tiled matmuls, lhsT = v slice (32 cols)
    and a constant-2.0 column block (denominator*2, folds the sigmoid 0.5)
  - gate: tanh on ScalarE (same ACT table set as exp), then
    gn2 = (tanh+1) * recip(2*denom) on DVE; rw = av * gn2
  - output projection back to [q, o] layout + output bias, DMA out.
"""

import sys

sys.path.insert(0, "/opt/trn_rl_repo")

import numpy as np
import ml_dtypes

import concourse.bass as bass
import concourse.mybir as mybir
import concourse.tile as tile
from concourse.bass_utils import run_bass_kernel_spmd

BF16 = mybir.dt.bfloat16
FP32 = mybir.dt.float32
F32R = mybir.dt.float32r

B, Q, KS, C, H, HD, OUT = 64, 512, 512, 256, 8, 32, 256
NCORES = 8
NB = B // NCORES  # batches per core = 8
KT = KS // 128  # 4 k-tiles
QT = Q // 128  # 4 q-tiles

# engine-split knob: which head-pairs get the bias-add on PE (identity
# matmul accumulate) vs on DVE (tensor_tensor on the way out of PSUM)
PE_ADD = lambda kt, pr: pr != 3  # noqa: E731

_CACHED = {}


def _split_multi_waits(nc, keep=1):
    """Walrus codegen only supports one sync-wait command on (at least)
    TensorTensor-class instructions. Move extra waits into standalone
    EventSemaphore instructions on the same engine queue, just before the
    offending instruction."""
    n = 0
    for f in nc.m.functions:
        for bb in f.blocks:
            out = []
            for ins in bb.instructions:
                si = ins.sync_info
                if si is not None and si.on_wait and len(si.on_wait) > keep:
                    waits = list(si.on_wait)
                    extra, last = waits[:-keep], waits[-keep:]
                    si.on_wait = last
                    for w in extra:
                        n += 1
                        wi = mybir.InstEventSemaphore(
                            name=f"WSPLIT-{n}",
                            engine=ins.engine,
                            ins=[],
                            outs=[],
                            sync_info=mybir.SyncInfo(on_wait=[w], on_update=[]),
                        )
                        out.append(wi)
                out.append(ins)
            bb.instructions = out
    return n


def _build_nc():
    nc = bass.Bass()
    # per-core inputs
    xq_d = nc.dram_tensor("xq", [NB, 128, 2, Q], F32R, kind="ExternalInput")
    xm_d = nc.dram_tensor("xm", [NB, 128, 2, KS], F32R, kind="ExternalInput")
    s12_d = nc.dram_tensor("s12", [NB, 128, KT, H, Q], BF16, kind="ExternalInput")
    wq_d = nc.dram_tensor("wq", [128, 2, C], F32R, kind="ExternalInput")
    wk_d = nc.dram_tensor("wk", [128, 2, C], F32R, kind="ExternalInput")
    wv_d = nc.dram_tensor("wv", [128, 2, C], F32R, kind="ExternalInput")
    wg_d = nc.dram_tensor("wg", [128, 2, C], F32R, kind="ExternalInput")
    ow_d = nc.dram_tensor("ow", [128, 2, OUT], F32R, kind="ExternalInput")
    gb_d = nc.dram_tensor("gb", [128, 2, 1], FP32, kind="ExternalInput")
    ob_d = nc.dram_tensor("ob", [128, OUT], FP32, kind="ExternalInput")
    id_d = nc.dram_tensor("ident", [128, 128], BF16, kind="ExternalInput")
    tw_d = nc.dram_tensor("twos", [128, 32], BF16, kind="ExternalInput")
    out_d = nc.dram_tensor("out", [NB, 128, QT, OUT], FP32, kind="ExternalOutput")

    with tile.TileContext(nc) as tc:
        with (
            tc.tile_pool(name="consts", bufs=1) as consts,
            tc.tile_pool(name="inp", bufs=2) as inp,
            tc.tile_pool(name="stage", bufs=2) as stage,
            tc.tile_pool(name="exw", bufs=5) as exw,
            tc.tile_pool(name="b12p", bufs=3) as b12p,
            tc.tile_pool(name="small", bufs=3) as small,
            tc.tile_pool(name="osbp", bufs=2) as osbp,
            tc.tile_pool(name="psmain", bufs=2, space="PSUM") as psmain,
            tc.tile_pool(name="psavd", bufs=2, space="PSUM") as psavd,
        ):
            # ---- constants ----
            wq_sb = consts.tile([128, 2, C], F32R, tag="wq")
            wk_sb = consts.tile([128, 2, C], F32R, tag="wk")
            wv_sb = consts.tile([128, 2, C], F32R, tag="wv")
            wg_sb = consts.tile([128, 2, C], F32R, tag="wg")
            ow_sb = consts.tile([128, 2, OUT], F32R, tag="ow")
            gb_sb = consts.tile([128, 2, 1], FP32, tag="gb")
            ob_sb = consts.tile([128, OUT], FP32, tag="ob")
            id_sb = consts.tile([128, 128], BF16, tag="ident")
            tw_sb = consts.tile([128, 32], BF16, tag="twos")
            for sb, d in (
                (wq_sb, wq_d), (wk_sb, wk_d), (wv_sb, wv_d), (wg_sb, wg_d),
                (ow_sb, ow_d), (gb_sb, gb_d), (ob_sb, ob_d), (id_sb, id_d),
                (tw_sb, tw_d),
            ):
                nc.sync.dma_start(sb[:], d[:])

            for b in range(NB):
                # ---- load per-batch inputs ----
                xq = inp.tile([128, 2, Q], F32R, tag="xq")
                xm = inp.tile([128, 2, KS], F32R, tag="xm")
                b12all = inp.tile([128, KT, H, Q], BF16, tag="b12all")
                nc.sync.dma_start(xq[:], xq_d[b])
                nc.sync.dma_start(xm[:], xm_d[b])
                nc.sync.dma_start(b12all[:], s12_d[b])

                # ---- projections ----
                qTs = stage.tile([128, 2, Q], F32R, tag="qTs")
                kTs = stage.tile([128, 2, KS], F32R, tag="kTs")
                gts = stage.tile([128, 2, Q], FP32, tag="gts")
                vs = stage.tile([128, KT, H * HD], BF16, tag="vs")  # [128,4,256]
                for half in range(2):
                    pq = psmain.tile([128, 2, 512], FP32, tag="lt")
                    for t in range(2):
                        nc.tensor.matmul(
                            pq[:, 0, :], (wq_sb[:, t, 128 * half:128 * half + 128]),
                            (xq[:, t, :]), start=(t == 0), stop=(t == 1))
                    nc.vector.tensor_copy(qTs[:, half, :], pq[:, 0, :])
                    pk = psmain.tile([128, 2, 512], FP32, tag="lt")
                    for t in range(2):
                        nc.tensor.matmul(
                            pk[:, 0, :], (wk_sb[:, t, 128 * half:128 * half + 128]),
                            (xm[:, t, :]), start=(t == 0), stop=(t == 1))
                    nc.vector.tensor_copy(kTs[:, half, :], pk[:, 0, :])
                    pg = psmain.tile([128, 2, 512], FP32, tag="lt")
                    for t in range(2):
                        nc.tensor.matmul(
                            pg[:, 0, :], (wg_sb[:, t, 128 * half:128 * half + 128]),
                            (xq[:, t, :]), start=(t == 0), stop=(t == 1))
                    # gate = sigmoid(x+gb) = 0.5*(1+tanh((x+gb)/2)); tanh here
                    nc.scalar.activation(
                        gts[:, half, :], pg[:, 0, :],
                        mybir.ActivationFunctionType.Tanh,
                        bias=gb_sb[:, half, :], scale=0.5)
                # v projection: v[k, hc]
                for kh in range(2):
                    pv = psmain.tile([128, 2, 512], FP32, tag="lt")
                    for j in range(2):
                        kt = 2 * kh + j
                        for t in range(2):
                            nc.tensor.matmul(
                                pv[:, j, :C],
                                (xm[:, t, 128 * kt:128 * kt + 128]),
                                (wv_sb[:, t, :]), start=(t == 0), stop=(t == 1))
                    nc.vector.tensor_copy(vs[:, 2 * kh:2 * kh + 2, :], pv[:, :, :C])

                # ---- logits^T, bias add, exp, AV + denominators ----
                ex = [None] * KT
                for kt in range(KT):
                    ex[kt] = exw.tile([128, H, Q], BF16, tag="ex", name="ex")
                avd = [None, None]
                for g in range(2):
                    avd[g] = psavd.tile([128, 2, 512], FP32, tag="avd", name="avd")
                for kt in range(KT):
                    for pr in range(4):
                        lt = psmain.tile([128, 2, 512], FP32, tag="lt")
                        b12 = b12all[:, kt, 2 * pr:2 * pr + 2, :]
                        pe_add = PE_ADD(kt, pr)
                        for j in range(2):
                            h = 2 * pr + j
                            band = 32 * (h % 4)
                            half = h // 4
                            nc.tensor.matmul(
                                lt[:, j, :],
                                (kTs[band:band + 32, half, 128 * kt:128 * kt + 128]),
                                (qTs[band:band + 32, half, :]),
                                start=True, stop=not pe_add,
                                tile_position=(band, 0))
                            if pe_add:
                                nc.tensor.matmul(
                                    lt[:, j, :], id_sb[:], b12[:, j, :],
                                    start=False, stop=True, skip_group_check=True)
                        if pe_add:
                            nc.scalar.activation(
                                ex[kt][:, 2 * pr:2 * pr + 2, :], lt[:],
                                mybir.ActivationFunctionType.Exp)
                        else:
                            lts = b12p.tile([128, 2, Q], FP32, tag="lts")
                            nc.vector.tensor_tensor(
                                lts[:], lt[:], b12[:], mybir.AluOpType.add)
                            nc.scalar.activation(
                                ex[kt][:, 2 * pr:2 * pr + 2, :], lts[:],
                                mybir.ActivationFunctionType.Exp)


                # ---- AV + denominators after all exps (keeps PE queue free
                # of head-of-line blocking on ACT) ----
                for h in range(H):
                    band = 32 * (h % 4)
                    g = h // 4
                    for kt in range(KT):
                        nc.tensor.matmul(
                            avd[g][band:band + 32, 0, :],
                            vs[:, kt, HD * h:HD * h + HD],
                            ex[kt][:, h, :],
                            start=(kt == 0), stop=(kt == KT - 1),
                            tile_position=(0, band))
                    for kt in range(KT):
                        nc.tensor.matmul(
                            avd[g][band:band + 32, 1, :],
                            tw_sb[:],
                            ex[kt][:, h, :],
                            start=(kt == 0), stop=(kt == KT - 1),
                            tile_position=(0, band))

                # ---- gating * 1/(2*denom), rw ----
                rw = stage.tile([128, 2, Q], F32R, tag="rw")
                for g in range(2):
                    rd = small.tile([128, Q], FP32, tag="rd")
                    nc.vector.reciprocal(rd[:], avd[g][:, 1, :])
                    gn2 = small.tile([128, Q], FP32, tag="gn2")
                    # (tanh + 1) * (1/(2*denom)) == sigmoid/denom
                    nc.vector.scalar_tensor_tensor(
                        gn2[:], gts[:, g, :], 1.0, rd[:],
                        mybir.AluOpType.add, mybir.AluOpType.mult)
                    nc.vector.tensor_tensor(
                        rw[:, g, :], avd[g][:, 0, :], gn2[:],
                        mybir.AluOpType.mult)

                # ---- output projection ----
                osb = osbp.tile([128, QT, OUT], FP32, tag="osb")
                for qt in range(QT):
                    po = psmain.tile([128, 2, 512], FP32, tag="lt")
                    for g in range(2):
                        nc.tensor.matmul(
                            po[:, 0, :OUT], (rw[:, g, 128 * qt:128 * qt + 128]),
                            (ow_sb[:, g, :]), start=(g == 0), stop=(g == 1))
                    nc.vector.tensor_tensor(
                        osb[:, qt, :], po[:, 0, :OUT], ob_sb[:],
                        mybir.AluOpType.add)
                nc.sync.dma_start(out_d[b], osb[:])

    nsplit = _split_multi_waits(nc)
    print(f"split {nsplit} multi-wait instructions")
    return nc


def _prep_host(q_data, m_data, bias, nonbatched_bias, query_w, key_w, value_w,
               gating_w, gating_b, output_w, output_b):
    bf = ml_dtypes.bfloat16
    f32 = np.float32

    def as_np(x, dt=f32):
        return np.ascontiguousarray(np.asarray(x), dtype=dt)

    q_data = as_np(q_data)
    m_data = as_np(m_data)
    bias = as_np(bias)
    nb = as_np(nonbatched_bias)

    # [B, C, Q] -> per batch [128, 2, Q]
    def xpose(x):
        t = x.transpose(0, 2, 1).reshape(B, 2, 128, x.shape[1])
        return np.ascontiguousarray(t.transpose(0, 2, 1, 3), dtype=f32)

    xq = xpose(q_data)  # [B, 128, 2, 512]
    xm = xpose(m_data)

    # s12[b, p, kt, h, q] = bias[b,0,q,kt*128+p] + nb[h,q,kt*128+p]
    # (combined on host in fp32 -> one bf16 rounding instead of two)
    nbt = nb.transpose(0, 2, 1).reshape(H, KT, 128, Q)  # [h, kt, p, q]
    s12 = np.empty((B, 128, KT, H, Q), dtype=bf)
    for b in range(B):
        bt = bias[b, 0].transpose(1, 0).reshape(KT, 128, Q)  # [kt, p, q]
        s12[b] = (bt[:, :, None, :] + nbt.transpose(1, 2, 0, 3)).astype(
            bf).transpose(1, 0, 2, 3)

    def wprep(w, scale=1.0):
        w2 = (as_np(w).reshape(C, -1) * scale).reshape(2, 128, -1)
        return np.ascontiguousarray(w2.transpose(1, 0, 2), dtype=f32)

    wq = wprep(query_w, HD ** -0.5)
    wk = wprep(key_w)
    wv = wprep(value_w)
    wg = wprep(gating_w)
    ow = wprep(output_w.reshape(C, OUT))
    gb = np.ascontiguousarray(
        (0.5 * as_np(gating_b).reshape(2, 128)[:, :, None]).transpose(1, 0, 2),
        dtype=f32)  # [128, 2, 1]
    ob = np.ascontiguousarray(
        np.broadcast_to(as_np(output_b), (128, OUT)), dtype=f32)
    ident = np.eye(128, dtype=bf)
    twos = np.full((128, 32), 2.0, dtype=bf)

    shared = dict(wq=wq, wk=wk, wv=wv, wg=wg, ow=ow, gb=gb, ob=ob,
                  ident=ident, twos=twos)
    in_maps = []
    for c in range(NCORES):
        s = slice(c * NB, (c + 1) * NB)
        m = dict(shared)
        m["xq"] = xq[s]
        m["xm"] = xm[s]
        m["s12"] = s12[s]
        in_maps.append(m)
    return in_maps


def kernel(_trace=False, **inputs):
    if "nc" not in _CACHED:
        _CACHED["nc"] = _build_nc()
    nc = _CACHED["nc"]
    in_maps = _prep_host(**inputs)
    res = run_bass_kernel_spmd(nc, in_maps, core_ids=list(range(NCORES)),
                               trace=_trace)
    _CACHED["last_results"] = res
    outs = [np.asarray(r["out"], dtype=np.float32) for r in res.results]
    # [NB, 128, QT, OUT] per core -> [B, Q, OUT]
    full = np.concatenate(outs, axis=0)  # [B, 128, QT, OUT]
    return np.ascontiguousarray(full.transpose(0, 2, 1, 3).reshape(B, Q, OUT))


if __name__ == "__main__":
    rng = np.random.default_rng(0)
    ins = {
        "q_data": rng.standard_normal((B, Q, C), dtype=np.float32),
        "m_data": rng.standard_normal((B, KS, C), dtype=np.float32),
        "bias": rng.standard_normal((B, 1, Q, KS), dtype=np.float32),
        "nonbatched_bias": rng.standard_normal((H, Q, KS), dtype=np.float32),
        "query_w": rng.standard_normal((C, H, HD), dtype=np.float32) * 0.05,
        "key_w": rng.standard_normal((C, H, HD), dtype=np.float32) * 0.05,
        "value_w": rng.standard_normal((C, H, HD), dtype=np.float32) * 0.05,
        "gating_w": rng.standard_normal((C, H, HD), dtype=np.float32) * 0.05,
        "gating_b": np.ones((H, HD), dtype=np.float32),
        "output_w": rng.standard_normal((H, HD, OUT), dtype=np.float32) * 0.05,
        "output_b": np.zeros((OUT,), dtype=np.float32),
    }
    out = kernel(**ins)
    print(out.shape, out.dtype, np.abs(out).mean())
